# revision 12
# baseline (speedup 1.0000x reference)
"""Trainium2 Bass kernel for nn_AbsoluteHallway (ring-buffer GRU scan).

Self-contained: takes FULL inputs, shards batch over 8 NeuronCores,
runs a 256-step recurrent scan with a DRAM ring buffer accessed via
indirect DMA gather/scatter, returns FULL [128, 1000] logits.
"""
import sys

sys.path.insert(0, '/opt/trn_rl_repo')

import numpy as np

import concourse.bass as bass
import concourse.bacc as bacc
import concourse.mybir as mybir
import concourse.tile as tile
from concourse.tile import add_dep_helper
from concourse.bass_utils import run_bass_kernel_spmd

F32 = mybir.dt.float32
I32 = mybir.dt.int32
Alu = mybir.AluOpType
Act = mybir.ActivationFunctionType

B, T, IN = 128, 256, 256
S, R, C = 64, 8192, 1000
NCORES = 8
BL = B // NCORES            # 16 samples per core
NW = 5 * BL                 # 80 window rows per step
NG = NW + BL                # 96 gather rows (80 ring + 16 theta)
THETA_BASE = BL * R
NROW = BL * R + R           # ring + theta rows, 64 f32 each

BIG = 1.0e9


def _host_consts():
    samp = np.arange(NW) // 5
    consts = {}
    consts["offs80"] = (np.arange(NW) % 5 - 2).astype(np.float32)[:, None]
    consts["base96"] = np.concatenate(
        [(samp * R).astype(np.float32), np.zeros(BL, np.float32)])[:, None]
    r5x = np.zeros((BL + 1, NW), np.float32)
    r5x[samp, np.arange(NW)] = 1.0
    consts["r5x"] = r5x                      # [17, 80] replication (row 16 = 0)
    mask = r5x[:BL].T.copy()                 # [80, 16]
    consts["mask80"] = mask
    consts["maskmean"] = (mask * 0.2).astype(np.float32)
    consts["ones1x80"] = np.ones((1, NW), np.float32)
    consts["ones80"] = np.ones((NW, 1), np.float32)
    consts["ident80"] = np.eye(NW, dtype=np.float32)
    # affine index assembly: idxx[96,1] = C96.T @ [p0_16; 1]
    c96 = np.zeros((BL + 1, NG), np.float32)
    c96[samp, np.arange(NW)] = 1.0                       # window rows: p0_i
    c96[BL, np.arange(NW)] = (np.arange(NW) % 5 - 2)     # + (k-2)
    c96[np.arange(BL), NW + np.arange(BL)] = 1.0         # theta rows: p0_i
    c96[BL, NW + np.arange(BL)] = float(THETA_BASE)      # + theta base
    consts["c96"] = c96
    consts["lo96"] = np.concatenate(
        [np.zeros(NW, np.float32), np.full(BL, -BIG, np.float32)])[:, None]
    consts["hi96"] = np.concatenate(
        [np.full(NW, float(R), np.float32),
         np.full(BL, BIG, np.float32)])[:, None]
    consts["cap96"] = np.concatenate(
        [np.full(NW, BIG, np.float32),
         np.full(BL, float(THETA_BASE + R - 1), np.float32)])[:, None]
    # selector to move gather rows 80:96 down to partitions 0:16
    sel = np.zeros((NG, BL), np.float32)
    sel[NW + np.arange(BL), np.arange(BL)] = 1.0
    consts["sel96"] = sel
    pp0 = np.zeros((BL + 1, 2), np.float32)
    pp0[BL, 0] = 1.0
    consts["pp0"] = pp0
    selm = np.zeros((NW, 5), np.float32)
    selm[np.arange(NW), np.arange(NW) % 5] = 1.0
    consts["selmask"] = selm
    return consts


_CONST_SPECS = {
    "offs80": [NW, 1], "base96": [NG, 1], "r5x": [BL + 1, NW],
    "mask80": [NW, BL], "maskmean": [NW, BL], "ones1x80": [1, NW],
    "ones80": [NW, 1], "ident80": [NW, NW], "c96": [BL + 1, NG],
    "lo96": [NG, 1], "hi96": [NG, 1], "cap96": [NG, 1], "sel96": [NG, BL],
    "pp0": [BL + 1, 2], "selmask": [NW, 5],
}


def build_program(nsteps=T, hw_alias=True, dbg=False):
    nc = bacc.Bacc(None, target_bir_lowering=False)

    xT = nc.dram_tensor("xT", [IN, BL * T], F32, kind="ExternalInput")
    theta_tab = nc.dram_tensor("theta_tab", [R, 2], F32, kind="ExternalInput")
    ip_w = nc.dram_tensor("ip_w", [S, IN], F32, kind="ExternalInput")
    wihT = nc.dram_tensor("wihT", [S, 3 * S], F32, kind="ExternalInput")
    whhT = nc.dram_tensor("whhT", [S, 3 * S], F32, kind="ExternalInput")
    jwT = nc.dram_tensor("jwT", [S, 1], F32, kind="ExternalInput")
    jb_in = nc.dram_tensor("jb", [1, 1], F32, kind="ExternalInput")
    bih_in = nc.dram_tensor("bih", [3 * S, 1], F32, kind="ExternalInput")
    bhh_in = nc.dram_tensor("bhh", [3 * S, 1], F32, kind="ExternalInput")
    ipb_in = nc.dram_tensor("ipb", [S, 1], F32, kind="ExternalInput")
    headT_aug = nc.dram_tensor("headT_aug", [S + 1, C], F32, kind="ExternalInput")
    cdram = {k: nc.dram_tensor("c_" + k, sh, F32, kind="ExternalInput")
             for k, sh in _CONST_SPECS.items()}
    logits_out = nc.dram_tensor("logits_out", [BL, C], F32, kind="ExternalOutput")
    ptr_dbg = nc.dram_tensor("ptr_dbg", [max(nsteps, 1), BL], F32,
                             kind="ExternalOutput") if dbg else None

    ring_w = nc.dram_tensor("ring_w", [NROW, S], F32)
    if hw_alias:
        ring_r = nc.dram_tensor("ring_r", [NROW, S], F32)
        nc.lookup_mls(ring_r).memorylocations[0].addr = \
            nc.lookup_mls(ring_w).memorylocations[0].addr
    else:
        ring_r = ring_w

    with tile.TileContext(nc) as tc:
        import contextlib
        with contextlib.ExitStack() as ctx:
            persist = ctx.enter_context(tc.tile_pool(name="persist", bufs=1))
            sb = ctx.enter_context(tc.tile_pool(name="work", bufs=3))
            sb2 = ctx.enter_context(tc.tile_pool(name="work2", bufs=3))
            ps = ctx.enter_context(tc.tile_pool(name="ps", bufs=5, space="PSUM"))
            psb = ctx.enter_context(tc.tile_pool(name="psb", bufs=3, space="PSUM"))

            # ---------- constants / weights ----------
            cs = {}
            for k, sh in _CONST_SPECS.items():
                cs[k] = persist.tile(sh, F32, name="cs_" + k, tag="cs_" + k)
                nc.sync.dma_start(out=cs[k][:], in_=cdram[k][:])
            wh = persist.tile([S, 3 * S], F32)
            wih_sb = persist.tile([S, 3 * S], F32)
            ipw_sb = persist.tile([S, IN], F32)
            jw_sb = persist.tile([S, 1], F32)
            for dst, src in [(wh, whhT), (wih_sb, wihT), (ipw_sb, ip_w),
                             (jw_sb, jwT)]:
                nc.sync.dma_start(out=dst[:], in_=src[:])

            # ---------- zero ring + theta fill ----------
            zsb = persist.tile([128, 2048], F32)
            nc.vector.memset(zsb[:], 0.0)
            init_ring_insts = []
            rows_per = 4096
            for i in range(NROW // rows_per):
                zi = nc.sync.dma_start(
                    out=ring_w[i * rows_per:(i + 1) * rows_per, :]
                    .rearrange("(a b) d -> a (b d)", a=128),
                    in_=zsb[:])
                init_ring_insts.append(zi)
            th_sb = persist.tile([128, 2 * R // 128], F32)
            nc.sync.dma_start(out=th_sb[:], in_=theta_tab[:])
            tf = nc.sync.dma_start(
                out=ring_w[THETA_BASE:THETA_BASE + R, 0:2], in_=th_sb[:])
            init_ring_insts.append(tf)

            # ---------- bias vectors ----------
            ipb_sb = persist.tile([S, 1], F32)
            nc.sync.dma_start(out=ipb_sb[:], in_=ipb_in[:])
            bvec = {}
            for nm, dram, off in [("bih_r", bih_in, 0), ("bih_z", bih_in, S),
                                  ("bih_n", bih_in, 2 * S),
                                  ("bhh_r", bhh_in, 0), ("bhh_z", bhh_in, S),
                                  ("bhh_n", bhh_in, 2 * S)]:
                tl = persist.tile([S, 1], F32, name=nm, tag=nm)
                nc.sync.dma_start(out=tl[:], in_=dram[off:off + S, :])
                bvec[nm] = tl
            wipb = {}
            for gname, gsl in [("r", slice(0, S)), ("z", slice(S, 2 * S)),
                               ("n", slice(2 * S, 3 * S))]:
                p = ps.tile([S, 1], F32, tag="pss")
                nc.tensor.matmul(p[:], lhsT=wih_sb[:, gsl], rhs=ipb_sb[:],
                                 start=True, stop=True)
                wipb[gname] = p
            c_r = persist.tile([S, 1], F32)
            c_z = persist.tile([S, 1], F32)
            c_inn = persist.tile([S, 1], F32)
            c_hn = persist.tile([S, 1], F32)
            for dst, wp, b1, b2 in [(c_r, wipb["r"], "bih_r", "bhh_r"),
                                    (c_z, wipb["z"], "bih_z", "bhh_z")]:
                nc.vector.tensor_tensor(out=dst[:], in0=wp[:],
                                        in1=bvec[b1][:], op=Alu.add)
                nc.vector.tensor_tensor(out=dst[:], in0=dst[:],
                                        in1=bvec[b2][:], op=Alu.add)
            nc.vector.tensor_tensor(out=c_inn[:], in0=wipb["n"][:],
                                    in1=bvec["bih_n"][:], op=Alu.add)
            nc.vector.tensor_copy(c_hn[:], bvec["bhh_n"][:])
            jb_sb = persist.tile([1, 1], F32)
            nc.sync.dma_start(out=jb_sb[:], in_=jb_in[:])
            ones16r = persist.tile([1, BL], F32)
            nc.vector.memset(ones16r[:], 1.0)
            jb16_ps = ps.tile([BL, 1], F32, tag="pss")
            nc.tensor.matmul(jb16_ps[:], lhsT=ones16r[:], rhs=jb_sb[:],
                             start=True, stop=True)
            jb16 = persist.tile([BL, 1], F32)
            nc.scalar.copy(jb16[:], jb16_ps[:])

            # ---------- A.T and GI precompute (per gate r/z/n) ----------
            at_a = persist.tile([128, 3 * S], F32)   # input-features 0:128
            at_b = persist.tile([128, 3 * S], F32)   # input-features 128:256
            for at_t, insl in [(at_a, slice(0, 128)), (at_b, slice(128, 256))]:
                at_ps = psb.tile([128, 3 * S], F32, tag="psB")
                nc.tensor.matmul(at_ps[:], lhsT=ipw_sb[:, insl], rhs=wih_sb[:],
                                 start=True, stop=True)
                nc.scalar.copy(at_t[:], at_ps[:])
            gi_t = {g: persist.tile([S, BL * T], F32, name="gi_" + g, tag="gi_" + g) for g in "rzn"}
            gsl_of = {"r": slice(0, S), "z": slice(S, 2 * S),
                      "n": slice(2 * S, 3 * S)}
            for cchunk in range(BL * T // 512):
                csl = slice(cchunk * 512, (cchunk + 1) * 512)
                xh = sb.tile([128, 512], F32, tag="xh")
                xl = sb.tile([128, 512], F32, tag="xl")
                nc.sync.dma_start(out=xh[:], in_=xT[0:128, csl])
                nc.sync.dma_start(out=xl[:], in_=xT[128:256, csl])
                for g in "rzn":
                    g_ps = psb.tile([S, 512], F32, tag="psB")
                    nc.tensor.matmul(g_ps[:], lhsT=at_a[:, gsl_of[g]],
                                     rhs=xh[:], start=True, stop=False)
                    nc.tensor.matmul(g_ps[:], lhsT=at_b[:, gsl_of[g]],
                                     rhs=xl[:], start=False, stop=True)
                    nc.scalar.copy(gi_t[g][:, csl], g_ps[:])

            # ---------- loop state ----------
            # ppaug [17, 2]: col0 = [p0f; 1], col1 = [ptr; *]
            ppaug = persist.tile([BL + 1, 2], F32)
            nc.vector.tensor_copy(ppaug[:], cs["pp0"][:])
            pi16 = persist.tile([BL, 1], I32)
            nc.vector.memset(pi16[:], 0)
            histT = persist.tile([NW, 5], F32)
            nc.vector.memset(histT[:], 0.0)

            def idx_and_prep(sbp, psp):
                """From ppaug, compute gather indices + weight machinery."""
                o = {}
                pr_ps = psp.tile([NW, 2], F32, tag="pss")
                nc.tensor.matmul(pr_ps[:], lhsT=cs["r5x"][:], rhs=ppaug[:],
                                 start=True, stop=True)
                ppe = sbp.tile([NW, 2], F32, tag="ppe")
                nc.scalar.copy(ppe[:], pr_ps[:])
                ix_ps = psp.tile([NG, 1], F32, tag="pss")
                nc.tensor.matmul(ix_ps[:], lhsT=cs["c96"][:],
                                 rhs=ppaug[:, 0:1], start=True, stop=True)
                mlo = sbp.tile([NG, 1], F32, tag="mlo")
                nc.vector.tensor_scalar(out=mlo[:], in0=ix_ps[:],
                                        scalar1=cs["lo96"][:], scalar2=float(R),
                                        op0=Alu.is_lt, op1=Alu.mult)
                mhi = sbp.tile([NG, 1], F32, tag="mhi")
                nc.vector.tensor_scalar(out=mhi[:], in0=ix_ps[:],
                                        scalar1=cs["hi96"][:],
                                        scalar2=-float(R),
                                        op0=Alu.is_ge, op1=Alu.mult)
                ixw = sbp.tile([NG, 1], F32, tag="ixw")
                nc.vector.tensor_tensor(out=ixw[:], in0=ix_ps[:], in1=mlo[:],
                                        op=Alu.add)
                nc.vector.tensor_tensor(out=ixw[:], in0=ixw[:], in1=mhi[:],
                                        op=Alu.add)
                gidxg = sbp.tile([NG, 1], F32, tag="gidxg")
                nc.vector.tensor_scalar(out=gidxg[:], in0=ixw[:],
                                        scalar1=cs["cap96"][:],
                                        scalar2=cs["base96"][:],
                                        op0=Alu.min, op1=Alu.add)
                idxT = sbp.tile([NG, 1], I32, tag="idxT")
                nc.vector.tensor_scalar(out=idxT[:], in0=gidxg[:], scalar1=0.0,
                                        scalar2=None, op0=Alu.add)
                d1 = sbp.tile([NW, 1], F32, tag="d1")
                nc.vector.tensor_tensor(out=d1[:], in0=ppe[:, 0:1],
                                        in1=ppe[:, 1:2], op=Alu.subtract)
                nc.vector.tensor_tensor(out=d1[:], in0=cs["offs80"][:],
                                        in1=d1[:], op=Alu.add)
                dsq = sbp.tile([NW, 1], F32, tag="dsq")
                nc.vector.tensor_tensor(out=dsq[:], in0=d1[:], in1=d1[:],
                                        op=Alu.mult)
                wt = sbp.tile([NW, 1], F32, tag="wt")
                nc.scalar.activation(out=wt[:], in_=dsq[:], func=Act.Exp,
                                     scale=-2.0)
                sums_ps = psp.tile([BL, 1], F32, tag="pss")
                nc.tensor.matmul(sums_ps[:], lhsT=cs["mask80"][:], rhs=wt[:],
                                 start=True, stop=True)
                s16 = sbp.tile([BL, 1], F32, tag="s16")
                nc.vector.reciprocal(s16[:], sums_ps[:])
                s80_ps = psp.tile([NW, 1], F32, tag="pss")
                nc.tensor.matmul(s80_ps[:], lhsT=cs["r5x"][0:BL, :],
                                 rhs=s16[:], start=True, stop=True)
                wn = sbp.tile([NW, 1], F32, tag="wn")
                nc.vector.tensor_tensor(out=wn[:], in0=wt[:], in1=s80_ps[:],
                                        op=Alu.mult)
                wsel = sbp.tile([NW, BL], F32, tag="wsel")
                nc.vector.tensor_tensor(out=wsel[:], in0=cs["mask80"][:],
                                        in1=wn[:].to_broadcast([NW, BL]),
                                        op=Alu.mult)
                cr_ps = psp.tile([1, NW], F32, tag="pss")
                nc.tensor.transpose(cr_ps[:], gidxg[0:NW, :], cs["ident80"][:])
                crow = sbp.tile([1, NW], F32, tag="crow")
                nc.scalar.copy(crow[:], cr_ps[:])
                rr_ps = psp.tile([NW, NW], F32, tag="pss")
                nc.tensor.matmul(rr_ps[:], lhsT=cs["ones1x80"][:], rhs=crow[:],
                                 start=True, stop=True)
                rr = sbp.tile([NW, NW], F32, tag="rr")
                nc.scalar.copy(rr[:], rr_ps[:])
                o.update(idxT=idxT, gidx=gidxg, wn=wn, wsel=wsel, rr=rr)
                return o

            cur = idx_and_prep(sb, ps)
            gprev = persist.tile([NG, 1], F32)
            nc.vector.memset(gprev[:], -1.0)
            rr_prev0 = persist.tile([NW, NW], F32)
            nc.vector.memset(rr_prev0[:], -1.0)
            s_prev = persist.tile([NW, S], F32)
            nc.vector.memset(s_prev[:], 0.0)
            prev = dict(gidx=gprev, rr=rr_prev0)
            scatters = []

            for t in range(nsteps):
                col = bass.ds(t * BL, BL)
                # ---- gather ----
                G = sb2.tile([NG, S], F32, tag="G")
                gi_inst = nc.gpsimd.indirect_dma_start(
                    out=G[:], out_offset=None, in_=ring_r[:],
                    in_offset=bass.IndirectOffsetOnAxis(
                        ap=cur["idxT"][:, 0:1], axis=0))
                if hw_alias:
                    if t == 0:
                        for zi in init_ring_insts:
                            add_dep_helper(gi_inst.ins, zi.ins, sync=True,
                                           reason="gather after ring init")
                    if len(scatters) >= 2:
                        add_dep_helper(gi_inst.ins, scatters[-2].ins,
                                       sync=True, reason="lag2 scatter->gather")

                # ---- correction prep ----
                mt = sb2.tile([NW, NW], F32, tag="mt")
                nc.vector.tensor_tensor(
                    out=mt[:], in0=prev["gidx"][0:NW, :].to_broadcast([NW, NW]),
                    in1=cur["rr"][:], op=Alu.is_equal)
                mm_ = sb2.tile([NW, NW], F32, tag="mm_")
                nc.vector.tensor_tensor(
                    out=mm_[:], in0=cur["gidx"][0:NW, :].to_broadcast([NW, NW]),
                    in1=prev["rr"][:], op=Alu.is_equal)
                any_ps = ps.tile([NW, 1], F32, tag="pss")
                nc.tensor.matmul(any_ps[:], lhsT=mt[:], rhs=cs["ones80"][:],
                                 start=True, stop=True)
                onem = sb.tile([NW, 1], F32, tag="onem")
                nc.vector.tensor_scalar(out=onem[:], in0=any_ps[:],
                                        scalar1=-1.0, scalar2=1.0,
                                        op0=Alu.mult, op1=Alu.add)
                mw_ps = ps.tile([NW, BL], F32, tag="pss")
                nc.tensor.matmul(mw_ps[:], lhsT=mm_[:], rhs=cur["wsel"][:],
                                 start=True, stop=True)
                mws = sb.tile([NW, BL], F32, tag="mws")
                nc.scalar.copy(mws[:], mw_ps[:])
                wsel2 = sb.tile([NW, BL], F32, tag="wsel2")
                nc.vector.tensor_tensor(
                    out=wsel2[:], in0=cur["wsel"][:],
                    in1=onem[:].to_broadcast([NW, BL]), op=Alu.mult)
                # ---- theta scalars ----
                th_ps = ps.tile([BL, 2], F32, tag="pss")
                nc.tensor.matmul(th_ps[:], lhsT=cs["sel96"][:], rhs=G[:, 0:2],
                                 start=True, stop=True)
                tgj = sb.tile([BL, 1], F32, tag="tgj")
                nc.vector.tensor_tensor(out=tgj[:], in0=th_ps[:, 1:2],
                                        in1=jb16[:], op=Alu.add)
                tgt = sb.tile([BL, 1], F32, tag="tgt")
                nc.scalar.activation(out=tgt[:], in_=th_ps[:, 0:1],
                                     func=Act.Sigmoid)
                walk = sb.tile([BL, 1], F32, tag="walk")
                nc.vector.tensor_scalar(out=walk[:], in0=ppaug[0:BL, 1:2],
                                        scalar1=1.0, scalar2=None, op0=Alu.add)
                dtw = sb.tile([BL, 1], F32, tag="dtw")
                nc.vector.tensor_scalar(out=dtw[:], in0=tgt[:],
                                        scalar1=float(R), scalar2=None,
                                        op0=Alu.mult)
                nc.vector.tensor_tensor(out=dtw[:], in0=dtw[:], in1=walk[:],
                                        op=Alu.subtract)

                # ---- critical chain ----
                read_ps = ps.tile([S, BL], F32, tag="pss")
                nc.tensor.matmul(read_ps[:], lhsT=G[0:NW, :], rhs=wsel2[:],
                                 start=True, stop=False)
                nc.tensor.matmul(read_ps[:], lhsT=s_prev[:], rhs=mws[:],
                                 start=False, stop=True)
                read_fm = sb.tile([S, BL], F32, tag="readfm")
                nc.scalar.copy(read_fm[:], read_ps[:])
                gpsums = {}
                for g in "rzn":
                    gp = ps.tile([S, BL], F32, tag="pss")
                    nc.tensor.matmul(gp[:], lhsT=wh[:, gsl_of[g]],
                                     rhs=read_fm[:], start=True, stop=True)
                    gpsums[g] = gp
                gr = sb.tile([S, BL], F32, tag="gr")
                nc.vector.tensor_tensor(out=gr[:], in0=gpsums["r"][:],
                                        in1=gi_t["r"][:, col], op=Alu.add)
                r_t = sb.tile([S, BL], F32, tag="rt")
                nc.scalar.activation(out=r_t[:], in_=gr[:], func=Act.Sigmoid,
                                     bias=c_r[:])
                gz = sb.tile([S, BL], F32, tag="gz")
                nc.vector.tensor_tensor(out=gz[:], in0=gpsums["z"][:],
                                        in1=gi_t["z"][:, col], op=Alu.add)
                z_t = sb.tile([S, BL], F32, tag="zt")
                nc.scalar.activation(out=z_t[:], in_=gz[:], func=Act.Sigmoid,
                                     bias=c_z[:])
                hn_s = sb.tile([S, BL], F32, tag="hns")
                nc.vector.tensor_tensor(out=hn_s[:], in0=gpsums["n"][:],
                                        in1=c_hn[:].to_broadcast([S, BL]),
                                        op=Alu.add)
                rhn = sb.tile([S, BL], F32, tag="rhn")
                nc.vector.tensor_tensor(out=rhn[:], in0=r_t[:], in1=hn_s[:],
                                        op=Alu.mult)
                nin = sb.tile([S, BL], F32, tag="nin")
                nc.vector.tensor_tensor(out=nin[:], in0=rhn[:],
                                        in1=gi_t["n"][:, col], op=Alu.add)
                n_t = sb.tile([S, BL], F32, tag="nt")
                nc.scalar.activation(out=n_t[:], in_=nin[:], func=Act.Tanh,
                                     bias=c_inn[:])
                rmn = sb.tile([S, BL], F32, tag="rmn")
                nc.vector.tensor_tensor(out=rmn[:], in0=read_fm[:], in1=n_t[:],
                                        op=Alu.subtract)
                zr = sb.tile([S, BL], F32, tag="zr")
                nc.vector.tensor_tensor(out=zr[:], in0=z_t[:], in1=rmn[:],
                                        op=Alu.mult)
                h_sb = sb.tile([S, BL], F32, tag="h")
                nc.vector.tensor_tensor(out=h_sb[:], in0=n_t[:], in1=zr[:],
                                        op=Alu.add)
                jump_ps = ps.tile([BL, 1], F32, tag="pss")
                nc.tensor.matmul(jump_ps[:], lhsT=h_sb[:], rhs=jw_sb[:],
                                 start=True, stop=True)
                gate = sb.tile([BL, 1], F32, tag="gate")
                nc.scalar.activation(out=gate[:], in_=jump_ps[:],
                                     func=Act.Sigmoid, bias=tgj[:])
                gm = sb.tile([BL, 1], F32, tag="gm")
                nc.vector.tensor_tensor(out=gm[:], in0=gate[:], in1=dtw[:],
                                        op=Alu.mult)
                npv = sb.tile([BL, 1], F32, tag="npv")
                nc.vector.tensor_tensor(out=npv[:], in0=walk[:], in1=gm[:],
                                        op=Alu.add)
                m8 = sb.tile([BL, 1], F32, tag="m8")
                nc.vector.tensor_scalar(out=m8[:], in0=npv[:],
                                        scalar1=float(R), scalar2=float(R),
                                        op0=Alu.is_ge, op1=Alu.mult)
                nc.vector.tensor_tensor(out=ppaug[0:BL, 1:2], in0=npv[:],
                                        in1=m8[:], op=Alu.subtract)
                nc.vector.tensor_scalar(out=pi16[:], in0=ppaug[0:BL, 1:2],
                                        scalar1=0.0, scalar2=None, op0=Alu.add)
                nc.vector.tensor_copy(ppaug[0:BL, 0:1], pi16[:])
                if dbg:
                    nc.sync.dma_start(out=ptr_dbg[t:t + 1, :],
                                      in_=ppaug[0:BL, 1:2])
                nxt = idx_and_prep(sb, ps)
                if t >= nsteps - 5:
                    j = t - (nsteps - 5)
                    hc_ps = ps.tile([NW, 2], F32, tag="pss")
                    nc.tensor.matmul(hc_ps[:], lhsT=cs["r5x"][:], rhs=ppaug[:],
                                     start=True, stop=True)
                    nc.scalar.copy(histT[:, j:j + 1], hc_ps[:, 0:1])

                # ---- scatter path ----
                diag1m = sb2.tile([NW, NW], F32, tag="diag1m")
                nc.vector.tensor_tensor(out=diag1m[:], in0=cs["ident80"][:],
                                        in1=onem[:].to_broadcast([NW, NW]),
                                        op=Alu.mult)
                gc_ps = psb.tile([NW, S], F32, tag="psB")
                nc.tensor.matmul(gc_ps[:], lhsT=mt[:], rhs=s_prev[:],
                                 start=True, stop=False)
                nc.tensor.matmul(gc_ps[:], lhsT=diag1m[:], rhs=G[0:NW, :],
                                 start=False, stop=True)
                gc = sb2.tile([NW, S], F32, tag="gc")
                nc.scalar.copy(gc[:], gc_ps[:])
                hsm_ps = ps.tile([BL, S], F32, tag="pss")
                nc.tensor.transpose(hsm_ps[:], h_sb[:], cs["ident80"][0:S, 0:S])
                h_sm = sb.tile([BL, S], F32, tag="hsm")
                nc.scalar.copy(h_sm[:], hsm_ps[:])
                h80_ps = psb.tile([NW, S], F32, tag="psB")
                nc.tensor.matmul(h80_ps[:], lhsT=cs["r5x"][0:BL, :],
                                 rhs=h_sm[:], start=True, stop=True)
                delta = sb2.tile([NW, S], F32, tag="delta")
                nc.vector.tensor_tensor(out=delta[:], in0=h80_ps[:], in1=gc[:],
                                        op=Alu.subtract)
                upd = sb2.tile([NW, S], F32, tag="upd")
                nc.vector.tensor_tensor(out=upd[:],
                                        in0=cur["wn"][:].to_broadcast([NW, S]),
                                        in1=delta[:], op=Alu.mult)
                s_new = sb2.tile([NW, S], F32, tag="snew")
                nc.vector.tensor_tensor(out=s_new[:], in0=gc[:], in1=upd[:],
                                        op=Alu.add)
                sc_inst = nc.gpsimd.indirect_dma_start(
                    out=ring_w[:],
                    out_offset=bass.IndirectOffsetOnAxis(
                        ap=cur["idxT"][0:NW, 0:1], axis=0),
                    in_=s_new[:], in_offset=None)
                scatters.append(sc_inst)
                s_prev = s_new
                prev = cur
                cur = nxt

            # ---------- tail: pooled readout + head ----------
            hsel = persist.tile([NW, 5], F32)
            nc.vector.tensor_tensor(out=hsel[:], in0=histT[:],
                                    in1=cs["selmask"][:], op=Alu.mult)
            hred = persist.tile([NW, 1], F32)
            nc.vector.tensor_reduce(out=hred[:], in_=hsel[:],
                                    axis=mybir.AxisListType.X, op=Alu.add)
            hidx = persist.tile([NW, 1], I32)
            nc.vector.tensor_scalar(out=hidx[:], in0=hred[:],
                                    scalar1=cs["base96"][0:NW, :],
                                    scalar2=None, op0=Alu.add)
            gh_t = persist.tile([NW, S], F32)
            ghi = nc.gpsimd.indirect_dma_start(
                out=gh_t[:], out_offset=None, in_=ring_r[:],
                in_offset=bass.IndirectOffsetOnAxis(ap=hidx[:, 0:1], axis=0))
            if hw_alias:
                for si in scatters[-2:]:
                    add_dep_helper(ghi.ins, si.ins, sync=True,
                                   reason="pooled gather after all scatters")
            pool_ps = ps.tile([S, BL], F32, tag="pss")
            nc.tensor.matmul(pool_ps[:], lhsT=gh_t[:], rhs=cs["maskmean"][:],
                             start=True, stop=True)
            pooled_aug = persist.tile([S + 1, BL], F32)
            nc.vector.memset(pooled_aug[:], 1.0)
            nc.scalar.copy(pooled_aug[0:S, :], pool_ps[:])
            headw_sb = persist.tile([S + 1, C], F32)
            nc.sync.dma_start(out=headw_sb[:], in_=headT_aug[:])
            logit_sb = persist.tile([BL, C], F32)
            for chunk in range(2):
                csl = slice(chunk * 500, (chunk + 1) * 500)
                lg_ps = psb.tile([BL, 500], F32, tag="psB")
                nc.tensor.matmul(lg_ps[:], lhsT=pooled_aug[:],
                                 rhs=headw_sb[:, csl], start=True, stop=True)
                nc.scalar.copy(logit_sb[:, csl], lg_ps[:])
            nc.sync.dma_start(out=logits_out[:], in_=logit_sb[:])

    nc.compile()
    return nc


def host_prep(inputs):
    consts = _host_consts()
    x = np.ascontiguousarray(inputs["x"], np.float32)
    theta_tab = np.stack([np.asarray(inputs["theta_ptr"], np.float32),
                          np.asarray(inputs["theta_gate"], np.float32)], 1)
    headT_a = np.concatenate(
        [np.asarray(inputs["head_w"], np.float32).T,
         np.asarray(inputs["head_b"], np.float32)[None, :]], 0)
    shared = {
        "theta_tab": np.ascontiguousarray(theta_tab),
        "ip_w": np.asarray(inputs["ip_w"], np.float32),
        "wihT": np.ascontiguousarray(np.asarray(inputs["gru_wih"], np.float32).T),
        "whhT": np.ascontiguousarray(np.asarray(inputs["gru_whh"], np.float32).T),
        "jwT": np.ascontiguousarray(np.asarray(inputs["jump_w"], np.float32).T),
        "jb": np.asarray(inputs["jump_b"], np.float32).reshape(1, 1),
        "bih": np.asarray(inputs["gru_bih"], np.float32).reshape(3 * S, 1),
        "bhh": np.asarray(inputs["gru_bhh"], np.float32).reshape(3 * S, 1),
        "ipb": np.asarray(inputs["ip_b"], np.float32).reshape(S, 1),
        "headT_aug": np.ascontiguousarray(headT_a),
    }
    for k in _CONST_SPECS:
        shared["c_" + k] = np.ascontiguousarray(consts[k], np.float32)
    in_maps = []
    for c in range(NCORES):
        xl = x[c * BL:(c + 1) * BL]
        xTl = np.ascontiguousarray(
            np.transpose(xl, (2, 1, 0)).reshape(IN, T * BL))
        in_maps.append({**shared, "xT": xTl})
    return in_maps


_CACHED = {}


def kernel(**inputs):
    if "prog" not in _CACHED:
        _CACHED["prog"] = build_program(nsteps=T, hw_alias=True)
    nc = _CACHED["prog"]
    in_maps = host_prep(inputs)
    res = run_bass_kernel_spmd(nc, in_maps, list(range(NCORES)))
    out = np.concatenate([r["logits_out"] for r in res.results], 0)
    return out.astype(np.float32)


# revision 14
# speedup vs baseline: 1.1263x; 1.1263x over previous
"""Trainium2 Bass kernel for nn_AbsoluteHallway (ring-buffer GRU scan).

Self-contained: takes FULL inputs, shards batch over 8 NeuronCores,
runs a 256-step recurrent scan with a DRAM ring buffer accessed via
indirect DMA gather/scatter, returns FULL [128, 1000] logits.

v2: sigmoid-only activations (no ACT table reloads), PSUM-accumulated
gate inputs, fused index arithmetic (int AND wrap), gather issued ahead
of the scatter on the gpsimd queue.
"""
import sys

sys.path.insert(0, '/opt/trn_rl_repo')

import numpy as np

import concourse.bass as bass
import concourse.bacc as bacc
import concourse.mybir as mybir
import concourse.tile as tile
from concourse.tile import add_dep_helper
from concourse.bass_utils import run_bass_kernel_spmd

F32 = mybir.dt.float32
I32 = mybir.dt.int32
Alu = mybir.AluOpType
Act = mybir.ActivationFunctionType

B, T, IN = 128, 256, 256
S, R, C = 64, 8192, 1000
NCORES = 8
BL = B // NCORES            # 16 samples per core
NW = 5 * BL                 # 80 window rows per step
NG = NW + BL                # 96 gather rows (80 ring + 16 theta)
THETA_BASE = BL * R
NROW = BL * R + R + 1       # ring + theta rows + clamp-dup row


def _host_consts():
    samp = np.arange(NW) // 5
    koff = (np.arange(NW) % 5 - 2).astype(np.float32)
    consts = {}
    consts["base96"] = (np.concatenate(
        [(samp * R).astype(np.float32), np.zeros(BL, np.float32)])[:, None],
        F32)
    r5x = np.zeros((BL + 1, NW), np.float32)
    r5x[samp, np.arange(NW)] = 1.0
    consts["r5x"] = (r5x, F32)               # [17, 80] replication
    mask = r5x[:BL].T.copy()                 # [80, 16]
    consts["mask80"] = (mask, F32)
    consts["maskmean"] = ((mask * 0.2).astype(np.float32), F32)
    consts["ones1x80"] = (np.ones((1, NW), np.float32), F32)
    consts["ones80"] = (np.ones((NW, 1), np.float32), F32)
    consts["ident80"] = (np.eye(NW, dtype=np.float32), F32)
    # affine index assembly: idxx[96,1] = C96.T @ [p0_16; 1; fr_pad]
    c96 = np.zeros((BL + 1, NG), np.float32)
    c96[samp, np.arange(NW)] = 1.0
    c96[BL, np.arange(NW)] = koff
    c96[np.arange(BL), NW + np.arange(BL)] = 1.0
    c96[BL, NW + np.arange(BL)] = float(THETA_BASE)
    consts["c96"] = (c96, F32)
    andm = np.concatenate(
        [np.full(NW, R - 1, np.int32), np.full(BL, 0x7FFFFFFF, np.int32)])
    consts["andm96"] = (andm[:, None], I32)
    sel = np.zeros((NG, BL), np.float32)
    sel[NW + np.arange(BL), np.arange(BL)] = 1.0
    consts["sel96"] = (sel, F32)
    pp0 = np.zeros((BL + 1, 3), np.float32)
    pp0[BL, 0] = 1.0
    consts["pp0"] = (pp0, F32)
    selm = np.zeros((NW, 5), np.float32)
    selm[np.arange(NW), np.arange(NW) % 5] = 1.0
    consts["selmask"] = (selm, F32)
    # softmax-form gaussian weights: l_k = 4k*fr - 2k^2 (fr = ptr - p0)
    consts["c4k"] = ((4.0 * koff)[:, None].astype(np.float32), F32)
    consts["cm2k2"] = ((-2.0 * koff * koff)[:, None].astype(np.float32), F32)
    consts["cneg4k"] = ((-4.0 * koff)[:, None].astype(np.float32), F32)
    consts["cp2k2"] = ((2.0 * koff * koff)[:, None].astype(np.float32), F32)
    return consts


_CONSTS = _host_consts()


def build_program(nsteps=T, hw_alias=True, dbg=False):
    nc = bacc.Bacc(None, target_bir_lowering=False)

    xT = nc.dram_tensor("xT", [IN, BL * T], F32, kind="ExternalInput")
    theta_tab = nc.dram_tensor("theta_tab", [R, 2], F32, kind="ExternalInput")
    ip_w = nc.dram_tensor("ip_w", [S, IN], F32, kind="ExternalInput")
    wihT = nc.dram_tensor("wihT", [S, 3 * S], F32, kind="ExternalInput")
    whhT = nc.dram_tensor("whhT", [S, 3 * S], F32, kind="ExternalInput")
    jwT = nc.dram_tensor("jwT", [S, 1], F32, kind="ExternalInput")
    jb_in = nc.dram_tensor("jb", [1, 1], F32, kind="ExternalInput")
    bih_in = nc.dram_tensor("bih", [3 * S, 1], F32, kind="ExternalInput")
    bhh_in = nc.dram_tensor("bhh", [3 * S, 1], F32, kind="ExternalInput")
    bhh_nT_in = nc.dram_tensor("bhh_nT", [1, S], F32, kind="ExternalInput")
    ipb_in = nc.dram_tensor("ipb", [S, 1], F32, kind="ExternalInput")
    headT_aug = nc.dram_tensor("headT_aug", [S + 1, C], F32,
                               kind="ExternalInput")
    cdram = {k: nc.dram_tensor("c_" + k, list(v.shape), dt,
                               kind="ExternalInput")
             for k, (v, dt) in _CONSTS.items()}
    logits_out = nc.dram_tensor("logits_out", [BL, C], F32,
                                kind="ExternalOutput")
    ptr_dbg = nc.dram_tensor("ptr_dbg", [max(nsteps, 1), BL], F32,
                             kind="ExternalOutput") if dbg else None

    ring_w = nc.dram_tensor("ring_w", [NROW, S], F32)
    if hw_alias:
        ring_r = nc.dram_tensor("ring_r", [NROW, S], F32)
        nc.lookup_mls(ring_r).memorylocations[0].addr = \
            nc.lookup_mls(ring_w).memorylocations[0].addr
    else:
        ring_r = ring_w

    with tile.TileContext(nc) as tc:
        import contextlib
        with contextlib.ExitStack() as ctx:
            persist = ctx.enter_context(tc.tile_pool(name="persist", bufs=1))
            sb = ctx.enter_context(tc.tile_pool(name="work", bufs=3))
            sb2 = ctx.enter_context(tc.tile_pool(name="work2", bufs=3))
            ps = ctx.enter_context(tc.tile_pool(name="ps", bufs=5,
                                                space="PSUM"))
            psb = ctx.enter_context(tc.tile_pool(name="psb", bufs=3,
                                                 space="PSUM"))

            # ---------- constants / weights ----------
            cs = {}
            for k, (v, dt) in _CONSTS.items():
                cs[k] = persist.tile(list(v.shape), dt, name="cs_" + k,
                                     tag="cs_" + k)
                nc.sync.dma_start(out=cs[k][:], in_=cdram[k][:])
            wh = persist.tile([S, 3 * S], F32)
            wih_sb = persist.tile([S, 3 * S], F32)
            ipw_sb = persist.tile([S, IN], F32)
            jw_sb = persist.tile([S, 1], F32)
            bhh_nT = persist.tile([1, S], F32)
            for dst, src in [(wh, whhT), (wih_sb, wihT), (ipw_sb, ip_w),
                             (jw_sb, jwT), (bhh_nT, bhh_nT_in)]:
                nc.sync.dma_start(out=dst[:], in_=src[:])

            # ---------- zero ring + theta fill ----------
            zsb = persist.tile([128, 2048], F32)
            nc.vector.memset(zsb[:], 0.0)
            init_ring_insts = []
            rows_per = 4096
            nchunks = (NROW + rows_per - 1) // rows_per
            for i in range(nchunks):
                lo = i * rows_per
                hi = min(lo + rows_per, NROW)
                n = hi - lo
                if n == rows_per:
                    out_ap = ring_w[lo:hi, :].rearrange(
                        "(a b) d -> a (b d)", a=128)
                else:
                    out_ap = ring_w[lo:hi, :]
                zi = nc.sync.dma_start(out=out_ap,
                                       in_=zsb[:, 0:n * S // 128]
                                       if n == rows_per else zsb[0:n, 0:S])
                init_ring_insts.append(zi)
            th_sb = persist.tile([128, 2 * R // 128], F32)
            nc.sync.dma_start(out=th_sb[:], in_=theta_tab[:])
            tf = nc.sync.dma_start(
                out=ring_w[THETA_BASE:THETA_BASE + R, 0:2], in_=th_sb[:])
            init_ring_insts.append(tf)
            # clamp-dup row: theta[8191] copied to row THETA_BASE + R
            tf2 = nc.sync.dma_start(
                out=ring_w[THETA_BASE + R:THETA_BASE + R + 1, 0:2],
                in_=th_sb[127:128, 126:128])
            init_ring_insts.append(tf2)

            # ---------- bias vectors ----------
            ipb_sb = persist.tile([S, 1], F32)
            nc.sync.dma_start(out=ipb_sb[:], in_=ipb_in[:])
            bvec = {}
            for nm, dram, off in [("bih_r", bih_in, 0), ("bih_z", bih_in, S),
                                  ("bih_n", bih_in, 2 * S),
                                  ("bhh_r", bhh_in, 0),
                                  ("bhh_z", bhh_in, S)]:
                tl = persist.tile([S, 1], F32, name=nm, tag=nm)
                nc.sync.dma_start(out=tl[:], in_=dram[off:off + S, :])
                bvec[nm] = tl
            gsl_of = {"r": slice(0, S), "z": slice(S, 2 * S),
                      "n": slice(2 * S, 3 * S)}
            wipb = {}
            for g in "rzn":
                p = ps.tile([S, 1], F32, tag="pss", name="wipb_" + g)
                nc.tensor.matmul(p[:], lhsT=wih_sb[:, gsl_of[g]],
                                 rhs=ipb_sb[:], start=True, stop=True)
                wipb[g] = p
            c_r = persist.tile([S, 1], F32)
            c_z = persist.tile([S, 1], F32)
            c_inn2 = persist.tile([S, 1], F32)
            for dst, wp, b1, b2 in [(c_r, wipb["r"], "bih_r", "bhh_r"),
                                    (c_z, wipb["z"], "bih_z", "bhh_z")]:
                nc.vector.tensor_tensor(out=dst[:], in0=wp[:],
                                        in1=bvec[b1][:], op=Alu.add)
                nc.vector.tensor_tensor(out=dst[:], in0=dst[:],
                                        in1=bvec[b2][:], op=Alu.add)
            # c_inn2 = 2*(Wih@ipb + bih)_n  (tanh-as-sigmoid needs 2x bias)
            nc.vector.tensor_tensor(out=c_inn2[:], in0=wipb["n"][:],
                                    in1=bvec["bih_n"][:], op=Alu.add)
            nc.vector.tensor_scalar(out=c_inn2[:], in0=c_inn2[:],
                                    scalar1=2.0, scalar2=None, op0=Alu.mult)
            jb_sb = persist.tile([1, 1], F32)
            nc.sync.dma_start(out=jb_sb[:], in_=jb_in[:])
            ones16r = persist.tile([1, BL], F32)
            nc.vector.memset(ones16r[:], 1.0)
            jb16_ps = ps.tile([BL, 1], F32, tag="pss")
            nc.tensor.matmul(jb16_ps[:], lhsT=ones16r[:], rhs=jb_sb[:],
                             start=True, stop=True)
            jb16 = persist.tile([BL, 1], F32)
            nc.scalar.copy(jb16[:], jb16_ps[:])

            # ---------- A.T and GI precompute (per gate r/z/n) ----------
            at_a = persist.tile([128, 3 * S], F32)
            at_b = persist.tile([128, 3 * S], F32)
            for at_t, insl in [(at_a, slice(0, 128)), (at_b, slice(128, 256))]:
                at_ps = psb.tile([128, 3 * S], F32, tag="psB")
                nc.tensor.matmul(at_ps[:], lhsT=ipw_sb[:, insl],
                                 rhs=wih_sb[:], start=True, stop=True)
                nc.scalar.copy(at_t[:], at_ps[:])
            gi_t = {g: persist.tile([S, BL * T], F32, name="gi_" + g,
                                    tag="gi_" + g) for g in "rzn"}
            for cchunk in range(BL * T // 512):
                csl = slice(cchunk * 512, (cchunk + 1) * 512)
                xh = sb.tile([128, 512], F32, tag="xh")
                xl = sb.tile([128, 512], F32, tag="xl")
                nc.sync.dma_start(out=xh[:], in_=xT[0:128, csl])
                nc.sync.dma_start(out=xl[:], in_=xT[128:256, csl])
                for g in "rzn":
                    g_ps = psb.tile([S, 512], F32, tag="psB")
                    nc.tensor.matmul(g_ps[:], lhsT=at_a[:, gsl_of[g]],
                                     rhs=xh[:], start=True, stop=False)
                    nc.tensor.matmul(g_ps[:], lhsT=at_b[:, gsl_of[g]],
                                     rhs=xl[:], start=False, stop=True)
                    nc.scalar.copy(gi_t[g][:, csl], g_ps[:])

            # ---------- loop state ----------
            # ppaug [17, 3]: col0 = [p0f; 1], col1 = [ptr; *], col2 = [fr; *]
            ppaug = persist.tile([BL + 1, 3], F32)
            nc.vector.tensor_copy(ppaug[:], cs["pp0"][:])
            pi16 = persist.tile([BL, 1], I32)
            nc.vector.memset(pi16[:], 0)
            histT = persist.tile([NW, 5], F32)
            nc.vector.memset(histT[:], 0.0)

            def idx_part(sbp, psp):
                """Critical: gather indices from ppaug col0."""
                ix_ps = psp.tile([NG, 1], F32, tag="pss", name="ix_ps")
                nc.tensor.matmul(ix_ps[:], lhsT=cs["c96"][:],
                                 rhs=ppaug[:, 0:1], start=True, stop=True)
                i0 = sbp.tile([NG, 1], I32, tag="i0")
                nc.vector.tensor_scalar(out=i0[:], in0=ix_ps[:], scalar1=0.0,
                                        scalar2=None, op0=Alu.add)
                i1 = sbp.tile([NG, 1], I32, tag="i1")
                nc.vector.tensor_scalar(out=i1[:], in0=i0[:],
                                        scalar1=cs["andm96"][:],
                                        scalar2=None, op0=Alu.bitwise_and)
                idxT = sbp.tile([NG, 1], I32, tag="idxT")
                nc.vector.tensor_scalar(out=idxT[:], in0=i1[:],
                                        scalar1=cs["base96"][:],
                                        scalar2=None, op0=Alu.add)
                return idxT

            def w_part(idxT, sbp, psp):
                """Flight-parallel: weights + correction-prep inputs."""
                o = {}
                gidxg = sbp.tile([NW, 1], F32, tag="gidxg")
                nc.vector.tensor_scalar(out=gidxg[:], in0=idxT[0:NW, :],
                                        scalar1=0.0, scalar2=None,
                                        op0=Alu.add)
                pr_ps = psp.tile([NW, 3], F32, tag="pss", name="pr_ps")
                nc.tensor.matmul(pr_ps[:], lhsT=cs["r5x"][:], rhs=ppaug[:],
                                 start=True, stop=True)
                sp = sbp.tile([NW, 1], F32, tag="sp")
                nc.scalar.activation(out=sp[:], in_=pr_ps[:, 2:3],
                                     func=Act.Sigmoid, scale=cs["c4k"][:],
                                     bias=cs["cm2k2"][:])
                sm = sbp.tile([NW, 1], F32, tag="sm")
                nc.scalar.activation(out=sm[:], in_=pr_ps[:, 2:3],
                                     func=Act.Sigmoid, scale=cs["cneg4k"][:],
                                     bias=cs["cp2k2"][:])
                smr = sbp.tile([NW, 1], F32, tag="smr")
                nc.vector.reciprocal(smr[:], sm[:])
                wt = sbp.tile([NW, 1], F32, tag="wt")
                nc.vector.tensor_tensor(out=wt[:], in0=sp[:], in1=smr[:],
                                        op=Alu.mult)
                sums_ps = psp.tile([BL, 1], F32, tag="pss", name="sums_ps")
                nc.tensor.matmul(sums_ps[:], lhsT=cs["mask80"][:], rhs=wt[:],
                                 start=True, stop=True)
                s16 = sbp.tile([BL, 1], F32, tag="s16")
                nc.vector.reciprocal(s16[:], sums_ps[:])
                s80_ps = psp.tile([NW, 1], F32, tag="pss", name="s80_ps")
                nc.tensor.matmul(s80_ps[:], lhsT=cs["r5x"][0:BL, :],
                                 rhs=s16[:], start=True, stop=True)
                wn = sbp.tile([NW, 1], F32, tag="wn")
                nc.vector.tensor_tensor(out=wn[:], in0=wt[:], in1=s80_ps[:],
                                        op=Alu.mult)
                wsel = sbp.tile([NW, BL], F32, tag="wsel")
                nc.vector.tensor_tensor(out=wsel[:], in0=cs["mask80"][:],
                                        in1=wn[:].to_broadcast([NW, BL]),
                                        op=Alu.mult)
                cr_ps = psp.tile([1, NW], F32, tag="pss", name="cr_ps")
                nc.tensor.transpose(cr_ps[:], gidxg[:], cs["ident80"][:])
                crow = sbp.tile([1, NW], F32, tag="crow")
                nc.scalar.copy(crow[:], cr_ps[:])
                rr_ps = psp.tile([NW, NW], F32, tag="pss", name="rr_ps")
                nc.tensor.matmul(rr_ps[:], lhsT=cs["ones1x80"][:],
                                 rhs=crow[:], start=True, stop=True)
                rr = sbp.tile([NW, NW], F32, tag="rr")
                nc.scalar.copy(rr[:], rr_ps[:])
                o.update(idxT=idxT, gidx=gidxg, wn=wn, wsel=wsel, rr=rr)
                return o

            idxT0 = idx_part(sb, ps)
            cur = w_part(idxT0, sb, ps)
            gprev = persist.tile([NW, 1], F32)
            nc.vector.memset(gprev[:], -1.0)
            rr_prev0 = persist.tile([NW, NW], F32)
            nc.vector.memset(rr_prev0[:], -1.0)
            s_prev = persist.tile([NW, S], F32)
            nc.vector.memset(s_prev[:], 0.0)
            prev = dict(gidx=gprev, rr=rr_prev0)
            scatters = []
            gathers = []

            def issue_gather(idxT, t):
                G = sb2.tile([NG, S], F32, tag="G", name="G")
                gi_inst = nc.gpsimd.indirect_dma_start(
                    out=G[:], out_offset=None, in_=ring_r[:],
                    in_offset=bass.IndirectOffsetOnAxis(
                        ap=idxT[:, 0:1], axis=0))
                if hw_alias:
                    if t == 0:
                        for zi in init_ring_insts:
                            add_dep_helper(gi_inst.ins, zi.ins, sync=True,
                                           reason="gather after ring init")
                    if len(scatters) >= 1 and t >= 2:
                        add_dep_helper(gi_inst.ins, scatters[t - 2].ins,
                                       sync=True,
                                       reason="lag2 scatter->gather")
                gathers.append(gi_inst)
                return G

            G = issue_gather(idxT0, 0)

            for t in range(nsteps):
                col = bass.ds(t * BL, BL)
                # ---- correction prep ----
                mt = sb2.tile([NW, NW], F32, tag="mt")
                nc.vector.tensor_tensor(
                    out=mt[:], in0=prev["gidx"][:].to_broadcast([NW, NW]),
                    in1=cur["rr"][:], op=Alu.is_equal)
                mm_ = sb2.tile([NW, NW], F32, tag="mm_")
                nc.vector.tensor_tensor(
                    out=mm_[:], in0=cur["gidx"][:].to_broadcast([NW, NW]),
                    in1=prev["rr"][:], op=Alu.is_equal)
                any_ps = ps.tile([NW, 1], F32, tag="pss", name="any_ps")
                nc.tensor.matmul(any_ps[:], lhsT=mt[:], rhs=cs["ones80"][:],
                                 start=True, stop=True)
                onem = sb.tile([NW, 1], F32, tag="onem")
                nc.vector.tensor_scalar(out=onem[:], in0=any_ps[:],
                                        scalar1=-1.0, scalar2=1.0,
                                        op0=Alu.mult, op1=Alu.add)
                mw_ps = ps.tile([NW, BL], F32, tag="pss", name="mw_ps")
                nc.tensor.matmul(mw_ps[:], lhsT=mm_[:], rhs=cur["wsel"][:],
                                 start=True, stop=True)
                mws = sb.tile([NW, BL], F32, tag="mws")
                nc.scalar.copy(mws[:], mw_ps[:])
                wsel2 = sb.tile([NW, BL], F32, tag="wsel2")
                nc.vector.tensor_tensor(
                    out=wsel2[:], in0=cur["wsel"][:],
                    in1=onem[:].to_broadcast([NW, BL]), op=Alu.mult)
                # ---- theta scalars ----
                th_ps = ps.tile([BL, 2], F32, tag="pss", name="th_ps")
                nc.tensor.matmul(th_ps[:], lhsT=cs["sel96"][:], rhs=G[:, 0:2],
                                 start=True, stop=True)
                tgj = sb.tile([BL, 1], F32, tag="tgj")
                nc.vector.tensor_tensor(out=tgj[:], in0=th_ps[:, 1:2],
                                        in1=jb16[:], op=Alu.add)
                tgt = sb.tile([BL, 1], F32, tag="tgt")
                nc.scalar.activation(out=tgt[:], in_=th_ps[:, 0:1],
                                     func=Act.Sigmoid)
                walk = sb.tile([BL, 1], F32, tag="walk")
                nc.vector.tensor_scalar(out=walk[:], in0=ppaug[0:BL, 1:2],
                                        scalar1=1.0, scalar2=None,
                                        op0=Alu.add)
                dtw = sb.tile([BL, 1], F32, tag="dtw")
                nc.vector.tensor_scalar(out=dtw[:], in0=tgt[:],
                                        scalar1=float(R), scalar2=None,
                                        op0=Alu.mult)
                nc.vector.tensor_tensor(out=dtw[:], in0=dtw[:], in1=walk[:],
                                        op=Alu.subtract)

                # ---- critical chain ----
                read_ps = ps.tile([S, BL], F32, tag="pss", name="read_ps")
                nc.tensor.matmul(read_ps[:], lhsT=G[0:NW, :], rhs=wsel2[:],
                                 start=True, stop=False)
                nc.tensor.matmul(read_ps[:], lhsT=s_prev[:], rhs=mws[:],
                                 start=False, stop=True)
                read_fm = sb.tile([S, BL], F32, tag="readfm")
                nc.vector.tensor_copy(read_fm[:], read_ps[:])
                gpsums = {}
                for g in "rzn":
                    gp = ps.tile([S, BL], F32, tag="pss", name="gp_" + g)
                    if g == "n":
                        nc.tensor.matmul(gp[:], lhsT=bhh_nT[:],
                                         rhs=ones16r[:], start=True,
                                         stop=False)
                    else:
                        nc.tensor.matmul(gp[:],
                                         lhsT=cs["ident80"][0:S, 0:S],
                                         rhs=gi_t[g][:, col], start=True,
                                         stop=False)
                    nc.tensor.matmul(gp[:], lhsT=wh[:, gsl_of[g]],
                                     rhs=read_fm[:], start=False, stop=True)
                    gpsums[g] = gp
                r_t = sb.tile([S, BL], F32, tag="rt")
                nc.scalar.activation(out=r_t[:], in_=gpsums["r"][:],
                                     func=Act.Sigmoid, bias=c_r[:])
                z_t = sb.tile([S, BL], F32, tag="zt")
                nc.scalar.activation(out=z_t[:], in_=gpsums["z"][:],
                                     func=Act.Sigmoid, bias=c_z[:])
                rhn = sb.tile([S, BL], F32, tag="rhn")
                nc.vector.tensor_tensor(out=rhn[:], in0=r_t[:],
                                        in1=gpsums["n"][:], op=Alu.mult)
                nin = sb.tile([S, BL], F32, tag="nin")
                nc.vector.tensor_tensor(out=nin[:], in0=rhn[:],
                                        in1=gi_t["n"][:, col], op=Alu.add)
                # n = tanh(nin + c_inn) = 2*sigmoid(2*nin + 2*c_inn) - 1
                nu = sb.tile([S, BL], F32, tag="nu")
                nc.scalar.activation(out=nu[:], in_=nin[:], func=Act.Sigmoid,
                                     scale=2.0, bias=c_inn2[:])
                n_t = sb.tile([S, BL], F32, tag="nt")
                nc.vector.tensor_scalar(out=n_t[:], in0=nu[:], scalar1=2.0,
                                        scalar2=-1.0, op0=Alu.mult,
                                        op1=Alu.add)
                rmn = sb.tile([S, BL], F32, tag="rmn")
                nc.vector.tensor_tensor(out=rmn[:], in0=read_fm[:],
                                        in1=n_t[:], op=Alu.subtract)
                zr = sb.tile([S, BL], F32, tag="zr")
                nc.vector.tensor_tensor(out=zr[:], in0=z_t[:], in1=rmn[:],
                                        op=Alu.mult)
                h_sb = sb.tile([S, BL], F32, tag="h")
                nc.vector.tensor_tensor(out=h_sb[:], in0=n_t[:], in1=zr[:],
                                        op=Alu.add)
                jump_ps = ps.tile([BL, 1], F32, tag="pss", name="jump_ps")
                nc.tensor.matmul(jump_ps[:], lhsT=h_sb[:], rhs=jw_sb[:],
                                 start=True, stop=True)
                gate = sb.tile([BL, 1], F32, tag="gate")
                nc.scalar.activation(out=gate[:], in_=jump_ps[:],
                                     func=Act.Sigmoid, bias=tgj[:])
                npv = sb.tile([BL, 1], F32, tag="npv")
                nc.vector.scalar_tensor_tensor(out=npv[:], in0=gate[:],
                                               scalar=dtw[:], in1=walk[:],
                                               op0=Alu.mult, op1=Alu.add)
                m8 = sb.tile([BL, 1], F32, tag="m8")
                nc.vector.tensor_scalar(out=m8[:], in0=npv[:],
                                        scalar1=float(R), scalar2=float(R),
                                        op0=Alu.is_ge, op1=Alu.mult)
                nc.vector.tensor_tensor(out=ppaug[0:BL, 1:2], in0=npv[:],
                                        in1=m8[:], op=Alu.subtract)
                nc.vector.tensor_scalar(out=pi16[:], in0=ppaug[0:BL, 1:2],
                                        scalar1=0.0, scalar2=None,
                                        op0=Alu.add)
                nc.vector.tensor_copy(ppaug[0:BL, 0:1], pi16[:])
                nc.vector.tensor_tensor(out=ppaug[0:BL, 2:3],
                                        in0=ppaug[0:BL, 1:2],
                                        in1=ppaug[0:BL, 0:1],
                                        op=Alu.subtract)
                if dbg:
                    nc.sync.dma_start(out=ptr_dbg[t:t + 1, :],
                                      in_=ppaug[0:BL, 1:2])
                idxTn = idx_part(sb, ps)
                if t + 1 < nsteps:
                    Gn = issue_gather(idxTn, t + 1)
                nxt = w_part(idxTn, sb, ps)
                if t >= nsteps - 5:
                    j = t - (nsteps - 5)
                    hc_ps = ps.tile([NW, 1], F32, tag="pss", name="hc_ps")
                    nc.tensor.matmul(hc_ps[:], lhsT=cs["r5x"][:],
                                     rhs=ppaug[:, 0:1], start=True, stop=True)
                    nc.scalar.copy(histT[:, j:j + 1], hc_ps[:])

                # ---- scatter path ----
                diag1m = sb2.tile([NW, NW], F32, tag="diag1m")
                nc.vector.tensor_tensor(out=diag1m[:], in0=cs["ident80"][:],
                                        in1=onem[:].to_broadcast([NW, NW]),
                                        op=Alu.mult)
                gc_ps = psb.tile([NW, S], F32, tag="psB", name="gc_ps")
                nc.tensor.matmul(gc_ps[:], lhsT=mt[:], rhs=s_prev[:],
                                 start=True, stop=False)
                nc.tensor.matmul(gc_ps[:], lhsT=diag1m[:], rhs=G[0:NW, :],
                                 start=False, stop=True)
                gc = sb2.tile([NW, S], F32, tag="gc")
                nc.scalar.copy(gc[:], gc_ps[:])
                hsm_ps = ps.tile([BL, S], F32, tag="pss", name="hsm_ps")
                nc.tensor.transpose(hsm_ps[:], h_sb[:],
                                    cs["ident80"][0:S, 0:S])
                h_sm = sb.tile([BL, S], F32, tag="hsm")
                nc.scalar.copy(h_sm[:], hsm_ps[:])
                h80_ps = psb.tile([NW, S], F32, tag="psB", name="h80_ps")
                nc.tensor.matmul(h80_ps[:], lhsT=cs["r5x"][0:BL, :],
                                 rhs=h_sm[:], start=True, stop=True)
                delta = sb2.tile([NW, S], F32, tag="delta")
                nc.vector.tensor_tensor(out=delta[:], in0=h80_ps[:],
                                        in1=gc[:], op=Alu.subtract)
                upd = sb2.tile([NW, S], F32, tag="upd")
                nc.vector.tensor_tensor(out=upd[:],
                                        in0=cur["wn"][:].to_broadcast([NW, S]),
                                        in1=delta[:], op=Alu.mult)
                s_new = sb2.tile([NW, S], F32, tag="snew")
                nc.vector.tensor_tensor(out=s_new[:], in0=gc[:], in1=upd[:],
                                        op=Alu.add)
                sc_inst = nc.gpsimd.indirect_dma_start(
                    out=ring_w[:],
                    out_offset=bass.IndirectOffsetOnAxis(
                        ap=cur["idxT"][0:NW, 0:1], axis=0),
                    in_=s_new[:], in_offset=None)
                scatters.append(sc_inst)
                s_prev = s_new
                prev = cur
                cur = nxt
                if t + 1 < nsteps:
                    G = Gn

            # ---------- tail ----------
            hsel = persist.tile([NW, 5], F32)
            nc.vector.tensor_tensor(out=hsel[:], in0=histT[:],
                                    in1=cs["selmask"][:], op=Alu.mult)
            hred = persist.tile([NW, 1], F32)
            nc.vector.tensor_reduce(out=hred[:], in_=hsel[:],
                                    axis=mybir.AxisListType.X, op=Alu.add)
            hidx = persist.tile([NW, 1], I32)
            nc.vector.tensor_scalar(out=hidx[:], in0=hred[:],
                                    scalar1=cs["base96"][0:NW, :],
                                    scalar2=None, op0=Alu.add)
            gh_t = persist.tile([NW, S], F32)
            ghi = nc.gpsimd.indirect_dma_start(
                out=gh_t[:], out_offset=None, in_=ring_r[:],
                in_offset=bass.IndirectOffsetOnAxis(ap=hidx[:, 0:1], axis=0))
            if hw_alias:
                for si in scatters[-2:]:
                    add_dep_helper(ghi.ins, si.ins, sync=True,
                                   reason="pooled gather after all scatters")
            pool_ps = ps.tile([S, BL], F32, tag="pss")
            nc.tensor.matmul(pool_ps[:], lhsT=gh_t[:], rhs=cs["maskmean"][:],
                             start=True, stop=True)
            pooled_aug = persist.tile([S + 1, BL], F32)
            nc.vector.memset(pooled_aug[:], 1.0)
            nc.scalar.copy(pooled_aug[0:S, :], pool_ps[:])
            headw_sb = persist.tile([S + 1, C], F32)
            nc.sync.dma_start(out=headw_sb[:], in_=headT_aug[:])
            logit_sb = persist.tile([BL, C], F32)
            for chunk in range(2):
                csl = slice(chunk * 500, (chunk + 1) * 500)
                lg_ps = psb.tile([BL, 500], F32, tag="psB")
                nc.tensor.matmul(lg_ps[:], lhsT=pooled_aug[:],
                                 rhs=headw_sb[:, csl], start=True, stop=True)
                nc.scalar.copy(logit_sb[:, csl], lg_ps[:])
            nc.sync.dma_start(out=logits_out[:], in_=logit_sb[:])

    nc.compile()
    return nc


def host_prep(inputs):
    x = np.ascontiguousarray(inputs["x"], np.float32)
    theta_tab = np.stack([np.asarray(inputs["theta_ptr"], np.float32),
                          np.asarray(inputs["theta_gate"], np.float32)], 1)
    headT_a = np.concatenate(
        [np.asarray(inputs["head_w"], np.float32).T,
         np.asarray(inputs["head_b"], np.float32)[None, :]], 0)
    bhh = np.asarray(inputs["gru_bhh"], np.float32)
    shared = {
        "theta_tab": np.ascontiguousarray(theta_tab),
        "ip_w": np.asarray(inputs["ip_w"], np.float32),
        "wihT": np.ascontiguousarray(
            np.asarray(inputs["gru_wih"], np.float32).T),
        "whhT": np.ascontiguousarray(
            np.asarray(inputs["gru_whh"], np.float32).T),
        "jwT": np.ascontiguousarray(np.asarray(inputs["jump_w"],
                                               np.float32).T),
        "jb": np.asarray(inputs["jump_b"], np.float32).reshape(1, 1),
        "bih": np.asarray(inputs["gru_bih"], np.float32).reshape(3 * S, 1),
        "bhh": bhh.reshape(3 * S, 1),
        "bhh_nT": np.ascontiguousarray(bhh[2 * S:3 * S].reshape(1, S)),
        "ipb": np.asarray(inputs["ip_b"], np.float32).reshape(S, 1),
        "headT_aug": np.ascontiguousarray(headT_a),
    }
    for k, (v, dt) in _CONSTS.items():
        shared["c_" + k] = np.ascontiguousarray(v)
    in_maps = []
    for c in range(NCORES):
        xl = x[c * BL:(c + 1) * BL]
        xTl = np.ascontiguousarray(
            np.transpose(xl, (2, 1, 0)).reshape(IN, T * BL))
        in_maps.append({**shared, "xT": xTl})
    return in_maps


_CACHED = {}


def kernel(**inputs):
    if "prog" not in _CACHED:
        _CACHED["prog"] = build_program(nsteps=T, hw_alias=True)
    nc = _CACHED["prog"]
    in_maps = host_prep(inputs)
    res = run_bass_kernel_spmd(nc, in_maps, list(range(NCORES)))
    out = np.concatenate([r["logits_out"] for r in res.results], 0)
    return out.astype(np.float32)
